# revision 1
# baseline (speedup 1.0000x reference)
"""AttnConv2d Trainium2 Bass kernel.

Reference computation (per image, batch B=16, C=64, H=W=96):
  kf = conv3x3(x1, w1); qf = conv3x3(x2, w2); vf = conv3x3(x1, w3)
  key/qry = stride-3 non-overlapping 3x3 patch unfold of kf/qf
  scores[k, c, d] = sum_l key[k][c, l] * qry[k][d, l]   (k = patch class 0..8)
  attn = softmax(scores^T flattened [d, c*9+k] / 24)
  out[d, x] = sum_{c,t} attn[d, c*9+t] * vf_pad[c, x + off(t)]

Sharding: pure data parallel, 2 images per NeuronCore across 8 cores.

Per-core program (matmul operands bf16, fp32 PSUM accumulation):
 - inputs staged as padded 2-copy layout [128, 98*98]: partitions 0-63 hold
   x_pad, partitions 64-127 hold x_pad shifted one padded row (+98), so a
   K=128 matmul computes two conv taps at once; 9 taps -> 6 matmuls (3 of
   them zero-padded in the lower half). All matmuls are uniform 128x64 PE
   mode; kf/qf run concurrently in the two PE column groups.
 - kf/qf conv outputs are emitted directly in patch-class-grouped pixel
   order via 3D rhs access patterns, so the score matmuls only need a
   per-class DMA transpose.
 - weight lhsT tiles are built on-device with PE transpose-mode from one
   contiguous staged load (avoids scatter-descriptor DMAs); input images are
   cast-loaded contiguously into a staging tail of the padded tile, then
   placed by a single DVE copy.
 - softmax fused on ACT (exp with per-row bias and accumulated row sum),
   attention kernels transposed via PE transpose-mode, then the output
   einsum runs as 6 more 128x64 matmuls per 4-row strip on the padded vf.
"""

import numpy as np

try:
    import concourse.bass as bass  # noqa: F401
except Exception:  # pragma: no cover - path fallback for fresh containers
    import sys

    for p in ("/opt/trn_rl_repo", "/root/.axon_site/_ro/trn_rl_repo"):
        if p not in sys.path:
            sys.path.append(p)
    import concourse.bass as bass  # noqa: F401

from contextlib import ExitStack

import concourse.mybir as mybir
import concourse.tile as tile
from concourse import bacc
from concourse.bass_utils import run_bass_kernel_spmd
from concourse.masks import make_identity

F32 = mybir.dt.float32
BF16 = mybir.dt.bfloat16

B, C, H, W = 16, 64, 96, 96
NCORES = 8
BPC = B // NCORES  # images per core
HP = H + 2  # padded row length
NPAD = HP * HP
NCHK = 4  # input load staged in quarter-image chunks
NPIX = H * W
NSTRIP = H // 3  # 32 grouped strips (3 rows = one patch-row)
VSTRIP = H // 4  # 24 spatial strips (4 rows, N=384)
AA = 9  # patch classes / taps
L = (H // 3) * (W // 3)  # 1024 patches
SCALE = 1.0 / 24.0  # 1/sqrt(64*9)

_CACHE = {}


def _build_program():
    nc = bacc.Bacc("TRN2", target_bir_lowering=False, debug=False)

    x1c = nc.dram_tensor("x1c", [BPC, C, H, W], F32, kind="ExternalInput")
    x2c = nc.dram_tensor("x2c", [BPC, C, H, W], F32, kind="ExternalInput")
    w1 = nc.dram_tensor("w1", [C, C, 3, 3], F32, kind="ExternalInput")
    w2 = nc.dram_tensor("w2", [C, C, 3, 3], F32, kind="ExternalInput")
    w3 = nc.dram_tensor("w3", [C, C, 3, 3], F32, kind="ExternalInput")
    yc = nc.dram_tensor("yc", [BPC, C, H, W], F32, kind="ExternalOutput")

    with ExitStack() as ctx:
        tc = ctx.enter_context(tile.TileContext(nc))

        wpool = ctx.enter_context(tc.tile_pool(name="wpool", bufs=1))
        xpool = ctx.enter_context(tc.tile_pool(name="xpool", bufs=2))
        kqpool = ctx.enter_context(tc.tile_pool(name="kqpool", bufs=2))
        tpool = ctx.enter_context(tc.tile_pool(name="tpool", bufs=9))
        apool = ctx.enter_context(tc.tile_pool(name="apool", bufs=2))
        opool = ctx.enter_context(tc.tile_pool(name="opool", bufs=2))

        pkq = ctx.enter_context(tc.tile_pool(name="pkq", bufs=2, space="PSUM"))
        pv = ctx.enter_context(tc.tile_pool(name="pv", bufs=2, space="PSUM"))
        po = ctx.enter_context(tc.tile_pool(name="po", bufs=2, space="PSUM"))
        ps = ctx.enter_context(tc.tile_pool(name="ps", bufs=2, space="PSUM"))

        # ---- constants: identity, weight lhsT tiles via PE transpose ----
        identf = wpool.tile([64, 64], F32, name="identf")
        make_identity(nc, identf)
        ident = wpool.tile([64, 64], BF16, name="ident")
        nc.vector.tensor_copy(ident, identf)

        def load_weights(wsrc, tag):
            """Build pair tiles [128,64] (rows di*64+c) and zero-padded single
            tiles (rows 0-63 = di=2) from one contiguous staged load."""
            wst = wpool.tile([64, 576], F32, name="wst", tag="wst", bufs=2)
            nc.sync.dma_start(out=wst, in_=wsrc.rearrange("m c a b -> m (c a b)"))
            wv = wst.rearrange("m (c a b) -> m a c b", c=64, a=3, b=3)
            pairs, singles = [], []
            for dj in range(3):
                wp = wpool.tile([128, C], BF16, name=f"W{tag}p{dj}")
                wg = wpool.tile([64, 128], F32, name="wg", tag="wg", bufs=2)
                nc.vector.tensor_copy(wg.rearrange("m (a c) -> m a c", a=2), wv[:, 0:2, :, dj])
                psw = ps.tile([128, 64], F32, name="psw", tag="ps")
                nc.tensor.transpose(psw, wg, identf)
                nc.vector.tensor_copy(wp, psw)
                ws = wpool.tile([128, C], BF16, name=f"W{tag}s{dj}")
                nc.vector.memset(ws[64:128, :], 0.0)
                psw2 = ps.tile([128, 64], F32, name="psw", tag="ps")
                nc.tensor.transpose(psw2[0:64, :], wv[:, 2, :, dj], identf)
                nc.vector.tensor_copy(ws[0:64, :], psw2[0:64, :])
                pairs.append(wp)
                singles.append(ws)
            return pairs, singles

        Wk, Wks = load_weights(w1, "k")
        Wq, Wqs = load_weights(w2, "q")
        Wv, Wvs = load_weights(w3, "v")

        def load_pair(i):
            """Load image i of x1c/x2c into 2-copy padded layouts, chunk
            loads interleaved so the first conv strips can start early.

            Contiguous cast-DMA into small staging chunks, then DVE placement
            into the padded rows (avoids scatter-descriptor DMAs)."""
            rows = H // NCHK
            CB = rows * HP
            tiles = []
            for xc, name in ((x1c, "x1p"), (x2c, "x2p")):
                xp = xpool.tile([128, NPAD], BF16, name=name, tag=name)
                v = xp.rearrange("p (h w) -> p h w", h=HP, w=HP)
                nc.vector.memset(v[0:64, 0:1, :], 0.0)
                nc.vector.memset(v[0:64, HP - 1 : HP, :], 0.0)
                nc.vector.memset(v[0:64, 1 : HP - 1, 0:1], 0.0)
                nc.vector.memset(v[0:64, 1 : HP - 1, HP - 1 : HP], 0.0)
                tiles.append((xc, xp, v))
            for g in range(NCHK):
                for xc, xp, v in tiles:
                    xcv = xc[i].rearrange("c (g h) w -> g c h w", g=NCHK)
                    st = xpool.tile(
                        [64, rows * W], BF16, name="xstage", tag="xstage", bufs=3
                    )
                    nc.gpsimd.dma_start(
                        out=st, in_=xcv[g].rearrange("c h w -> c (h w)")
                    )
                    nc.vector.tensor_copy(
                        out=v[0:64, 1 + g * rows : 1 + (g + 1) * rows, 1 : W + 1],
                        in_=st.rearrange("p (h w) -> p h w", h=rows, w=W),
                    )
                # lower copy chunk g (shifted one padded row); chunk g needs
                # the first row of placement g+1, so copy the previous chunk
                if g > 0:
                    for xc, xp, v in tiles:
                        nc.sync.dma_start(
                            out=xp[64:128, (g - 1) * CB : g * CB],
                            in_=xp[0:64, HP + (g - 1) * CB : HP + g * CB],
                        )
            for xc, xp, v in tiles:
                nc.sync.dma_start(
                    out=xp[64:128, (NCHK - 1) * CB : NPAD - HP],
                    in_=xp[0:64, HP + (NCHK - 1) * CB : NPAD],
                )
                nc.vector.memset(xp[64:128, NPAD - HP : NPAD], 0.0)
            return tiles[0][1], tiles[1][1]

        # ------------------------------------------------------------------
        # per-image phases
        # ------------------------------------------------------------------
        def phase_A_kq(i, x1p, x2p):
            """kf+qf convs in grouped pixel order, PE column-group paired."""
            kq = kqpool.tile([128, AA * L], BF16, name="kq", tag="kq")
            x1v = x1p.rearrange("p (h w) -> p h w", h=HP, w=HP)
            x2v = x2p.rearrange("p (h w) -> p h w", h=HP, w=HP)
            kqv = kq.rearrange("p (ki kj l) -> p ki kj l", ki=3, kj=3, l=L)

            for r in range(NSTRIP):
                h0 = 3 * r

                def gr(xv, hs, dj):
                    return xv[:, h0 + hs : h0 + hs + 3, dj : dj + W].rearrange(
                        "p ki (b kj) -> p ki kj b", kj=3
                    )

                psum_kq = pkq.tile([128, 288], F32, name="psum_kq")
                pkv = psum_kq.rearrange("p (a b c) -> p a b c", a=3, b=3)
                # interleave kf (cols 0-63) and qf (cols 64-127): PE column
                # groups run them concurrently
                for j in range(3):
                    nc.tensor.matmul(
                        pkv[0:64], Wk[j], gr(x1v, 0, j), start=(j == 0), stop=False
                    )
                    nc.tensor.matmul(
                        pkv[64:128], Wq[j], gr(x2v, 0, j), start=(j == 0), stop=False
                    )
                for j in range(3):
                    nc.tensor.matmul(
                        pkv[0:64], Wks[j], gr(x1v, 2, j), start=False, stop=(j == 2)
                    )
                    nc.tensor.matmul(
                        pkv[64:128], Wqs[j], gr(x2v, 2, j), start=False, stop=(j == 2)
                    )
                nc.scalar.copy(out=kqv[:, :, :, 32 * r : 32 * r + 32], in_=pkv)
            return kq

        def vfp_alloc():
            vfp = xpool.tile([128, NPAD], BF16, name="vfp", tag="vfp")
            vv = vfp.rearrange("p (h w) -> p h w", h=HP, w=HP)
            nc.vector.memset(vv[0:64, 0:1, :], 0.0)
            nc.vector.memset(vv[0:64, HP - 1 : HP, :], 0.0)
            nc.vector.memset(vv[0:64, 1 : HP - 1, 0:1], 0.0)
            nc.vector.memset(vv[0:64, 1 : HP - 1, HP - 1 : HP], 0.0)
            return vfp

        def vf_strip(x1p, vfp, r):
            """one 4-row vf conv strip (PE column group 0)."""
            x1v = x1p.rearrange("p (h w) -> p h w", h=HP, w=HP)
            vv = vfp.rearrange("p (h w) -> p h w", h=HP, w=HP)
            h0 = 4 * r
            psum_v = pv.tile([64, 384], F32, name="psum_v")
            pvv = psum_v.rearrange("p (a c) -> p a c", a=4)
            for j in range(3):
                nc.tensor.matmul(
                    pvv, Wv[j], x1v[:, h0 : h0 + 4, j : j + W],
                    start=(j == 0), stop=False,
                )
            for j in range(3):
                nc.tensor.matmul(
                    pvv, Wvs[j], x1v[:, h0 + 2 : h0 + 6, j : j + W],
                    start=False, stop=(j == 2),
                )
            nc.vector.tensor_copy(out=vv[0:64, h0 + 1 : h0 + 5, 1 : W + 1], in_=pvv)

        def vfp_bottom(vfp):
            # gpsimd (SWDGE) so it does not serialize against DMA transposes
            CB = (H // NCHK) * HP
            for g in range(NCHK):
                nc.gpsimd.dma_start(
                    out=vfp[64:128, g * CB : (g + 1) * CB],
                    in_=vfp[0:64, HP + g * CB : HP + (g + 1) * CB],
                )
            nc.gpsimd.dma_start(
                out=vfp[64:128, NCHK * CB : NPAD - HP],
                in_=vfp[0:64, HP + NCHK * CB : NPAD],
            )
            nc.vector.memset(vfp[64:128, NPAD - HP : NPAD], 0.0)

        def phase_B_transpose(i, kq):
            kT = {}
            qT = {}
            for k in range(AA):
                kT[k] = tpool.tile([128, 8, 64], BF16, name="kT", tag="kT")
                nc.sync.dma_start_transpose(kT[k], kq[0:64, L * k : L * (k + 1)])
                qT[k] = tpool.tile([128, 8, 64], BF16, name="qT", tag="qT")
                nc.sync.dma_start_transpose(qT[k], kq[64:128, L * k : L * (k + 1)])
            return kT, qT

        def phase_B_rest(i, kT, qT):
            """scores, softmax, attn kernel transpose."""
            sc = apool.tile([64, AA, 64], F32, name="sc", bufs=1)

            def score_class(k, out_ps):
                for s in range(8):
                    nc.tensor.matmul(
                        out_ps,
                        qT[k][:, s, :],
                        kT[k][:, s, :],
                        start=(s == 0),
                        stop=(s == 7),
                    )

            for j in range(4):
                psum_s = ps.tile([128, 64], F32, name="psum_s", tag="ps")
                score_class(2 * j, psum_s[0:64])
                score_class(2 * j + 1, psum_s[64:128])
                nc.vector.tensor_copy(out=sc[:, 2 * j, :], in_=psum_s[0:64])
                nc.vector.tensor_copy(out=sc[:, 2 * j + 1, :], in_=psum_s[64:128])
            psum_s = ps.tile([128, 64], F32, name="psum_s", tag="ps")
            score_class(8, psum_s[0:64])
            nc.vector.tensor_copy(out=sc[:, 8, :], in_=psum_s[0:64])

            # softmax over 576 (free dim), fused exp+sum on ACT
            scf = sc.rearrange("p a c -> p (a c)")
            mx = apool.tile([64, 1], F32, name="mx", bufs=1)
            nc.vector.reduce_max(mx, scf, axis=mybir.AxisListType.X)
            nb = apool.tile([64, 1], F32, name="nb", bufs=1)
            nc.vector.tensor_scalar_mul(nb, mx, -SCALE)
            ex = apool.tile([64, 576], F32, name="ex", bufs=1)
            sm = apool.tile([64, 1], F32, name="sm", bufs=1)
            nc.scalar.activation(
                out=ex,
                in_=scf,
                func=mybir.ActivationFunctionType.Exp,
                bias=nb,
                scale=SCALE,
                accum_out=sm,
            )
            rs = apool.tile([64, 1], F32, name="rs", bufs=1)
            nc.vector.reciprocal(rs, sm)

            # normalize + permute taps into transpose-friendly order:
            # 64-blocks [t0 t3 t1 t4 t2 t5 | t6 t7 t8]
            attnP = apool.tile([64, 576], BF16, name="attnP", bufs=1)
            in1 = ex[:, 0:384].rearrange("p (bb a c) -> p a bb c", bb=2, a=3, c=64)
            out1 = attnP[:, 0:384].rearrange("p (a bb c) -> p a bb c", a=3, bb=2, c=64)
            nc.vector.tensor_scalar_mul(out1, in1, rs)
            nc.vector.tensor_scalar_mul(attnP[:, 384:576], ex[:, 384:576], rs)

            # transpose attnP into lhsT blocks [128, 6, 64]:
            # j<3: rows = taps (j, j+3); block3 = [t6|0], block4 = [0|t7],
            # block5 = [t8|0]  (zero-padded halves)
            attnT = apool.tile([128, 6, 64], BF16, name="attnT", bufs=1)
            nc.vector.memset(attnT[:, 3:6, :], 0.0)
            for j in range(4):
                psum_t = ps.tile([128, 64], BF16, name="psum_t", tag="ps")
                nc.tensor.transpose(psum_t, attnP[:, 128 * j : 128 * (j + 1)], ident)
                if j < 3:
                    nc.vector.tensor_copy(out=attnT[:, j, :], in_=psum_t)
                else:
                    nc.vector.tensor_copy(out=attnT[0:64, 3, :], in_=psum_t[0:64])
                    nc.vector.tensor_copy(out=attnT[64:128, 4, :], in_=psum_t[64:128])
            psum_t = ps.tile([128, 64], BF16, name="psum_t", tag="ps")
            nc.tensor.transpose(psum_t[0:64, :], attnP[:, 512:576], ident)
            nc.vector.tensor_copy(out=attnT[0:64, 5, :], in_=psum_t[0:64, :])
            return attnT

        def c_strip(i, attnT, vfp, r, state):
            """one output-einsum strip (PE column group 1, pairs with vf)."""
            vv = vfp.rearrange("p (h w) -> p h w", h=HP, w=HP)
            slots = [(0, 0, 0), (1, 0, 1), (2, 0, 2), (3, 2, 0), (4, 1, 1), (5, 2, 2)]
            q, rr = r // 3, r % 3
            if rr == 0:
                state["outb"] = opool.tile([128, 3, 384], F32, name="outb", tag="outb")
            h0 = 4 * r
            psum_o = po.tile([128, 384], F32, name="psum_o")
            pov = psum_o[64:128].rearrange("p (a c) -> p a c", a=4)
            for si, (jb, hs, dj) in enumerate(slots):
                nc.tensor.matmul(
                    pov,
                    attnT[:, jb, :],
                    vv[:, h0 + hs : h0 + hs + 4, dj : dj + W],
                    start=(si == 0),
                    stop=(si == 5),
                )
            nc.vector.tensor_copy(
                out=state["outb"][64:128, rr, :], in_=psum_o[64:128]
            )
            if rr == 2:
                yv = yc[i].rearrange("c h w -> c (h w)")
                nc.scalar.dma_start(
                    out=yv[:, q * 1152 : (q + 1) * 1152],
                    in_=state["outb"][64:128].rearrange("p a x -> p (a x)"),
                )

        # ------------------------------------------------------------------
        # schedule: kq0; vf0; kq1 (transposes0 overlap); B0; vf1||C0
        # interleaved (PE column groups 0/1); B1; C1
        # ------------------------------------------------------------------
        x1p0, x2p0 = load_pair(0)
        kq0 = phase_A_kq(0, x1p0, x2p0)
        x1p1, x2p1 = load_pair(1)
        vfp0 = vfp_alloc()
        for r in range(VSTRIP):
            vf_strip(x1p0, vfp0, r)
        vfp_bottom(vfp0)
        kT0, qT0 = phase_B_transpose(0, kq0)
        kq1 = phase_A_kq(1, x1p1, x2p1)
        kT1, qT1 = phase_B_transpose(1, kq1)
        attnT0 = phase_B_rest(0, kT0, qT0)
        vfp1 = vfp_alloc()
        cstate = {}
        for r in range(VSTRIP):
            vf_strip(x1p1, vfp1, r)
            c_strip(0, attnT0, vfp0, r, cstate)
        vfp_bottom(vfp1)
        attnT1 = phase_B_rest(1, kT1, qT1)
        for r in range(VSTRIP):
            c_strip(1, attnT1, vfp1, r, cstate)

    nc.compile()
    return nc


def _get_program():
    if "nc" not in _CACHE:
        _CACHE["nc"] = _build_program()
    return _CACHE["nc"]


def kernel(x1, x2, w1, w2, w3, **kwargs):
    x1 = np.ascontiguousarray(np.asarray(x1, dtype=np.float32))
    x2 = np.ascontiguousarray(np.asarray(x2, dtype=np.float32))
    w1 = np.ascontiguousarray(np.asarray(w1, dtype=np.float32))
    w2 = np.ascontiguousarray(np.asarray(w2, dtype=np.float32))
    w3 = np.ascontiguousarray(np.asarray(w3, dtype=np.float32))

    nc = _get_program()
    in_maps = [
        {
            "x1c": x1[i * BPC : (i + 1) * BPC],
            "x2c": x2[i * BPC : (i + 1) * BPC],
            "w1": w1,
            "w2": w2,
            "w3": w3,
        }
        for i in range(NCORES)
    ]
    try:
        res = run_bass_kernel_spmd(
            nc, in_maps, core_ids=list(range(NCORES)), **kwargs
        )
    except Exception:
        # one retry: transient device state can fail a first attempt
        res = run_bass_kernel_spmd(
            nc, in_maps, core_ids=list(range(NCORES)), **kwargs
        )
    out = np.concatenate([r["yc"] for r in res.results], axis=0)
    if kwargs:
        return out.astype(np.float32), res
    return out.astype(np.float32)

